# revision 1
# baseline (speedup 1.0000x reference)
"""BasicBlock kernel, 1D-Winograd F(2,3) variant.

Each 3x3 conv = x-direction Winograd F(2,3) (4 planes, 2 outputs per tile)
x y-direction direct (3 dy taps):

  V(j)[c,y,tx]  = B-combos of x[c, y, 2tx+b]          (gpsimd, 4 tensor ops)
  M(j)          = sum_{dy,ci} W'(dy,j)^T @ V(j)       (PE, 24 matmuls/psum-pair)
  out[...,2tx]   = M0+M1+M2,  out[...,2tx+1] = M1-M2-M3  (DVE reduce+stt chain)

PE streams 2/3 of the direct conv's columns.  PSUM plane pairs: tileA=[M1,M2],
tileB=[M0,M3], each one bank; combinations need only one PSUM operand per op:
  rA = reduce(M1+M2); u0 = M0 + rA; d = rA - 2*M2; u1 = d - M3.
"""

import os
from contextlib import ExitStack

import numpy as np

import concourse.bass as bass
import concourse.tile as tile
from concourse import bacc, mybir
from concourse.bass_utils import run_bass_kernel_spmd

F32 = mybir.dt.float32
F16 = mybir.dt.float16

N_CORES = 8
C = 256
H = W = 32
P = 128
CB = C // P
HP = H + 2
WP = W + 2
PAD = HP * WP
TX = W // 2          # 16 winograd column pairs
NPL = 4              # planes
HALF = (H // 2) * W  # 512
NIMG = 64 // N_CORES

XR = 3
HR = 2
# plane -> (pair tile key, offset): tileA=[M1,M2], tileB=[M0,M3]; one reduce
# over tileA feeds both output chains (cheapest DVE mix).
PLANE_SLOT = {1: ("A", 0), 2: ("A", 256), 0: ("B", 0), 3: ("B", 256)}
# matmul emission / weight storage order: j=1 first (first consumed)
JORD = (1, 2, 0, 3)
JPOS = {j: q for q, j in enumerate(JORD)}


def build(nimg: int = NIMG) -> bacc.Bacc:
    nc = bacc.Bacc("TRN2", target_bir_lowering=False, debug=False, enable_asserts=True)

    x_d = nc.dram_tensor("xp", [nimg, CB, P, PAD], F16, kind="ExternalInput")
    vx_d = nc.dram_tensor("vxp", [nimg, CB, P, NPL * HP * TX], F16, kind="ExternalInput")
    w1_d = nc.dram_tensor("w1t", [CB, P, 3 * NPL * CB * P], F16, kind="ExternalInput")
    w2_d = nc.dram_tensor("w2t", [CB, P, 3 * NPL * CB * P], F16, kind="ExternalInput")
    bn_d = nc.dram_tensor("bnv", [P, 4 * CB], F32, kind="ExternalInput")
    y_d = nc.dram_tensor("y", [nimg, C, H, W], F32, kind="ExternalOutput")

    with tile.TileContext(nc) as tc, ExitStack() as ctx:
        wpool = ctx.enter_context(tc.tile_pool(name="weights", bufs=1))
        xpool = ctx.enter_context(tc.tile_pool(name="xpad", bufs=XR))
        vxpool = ctx.enter_context(tc.tile_pool(name="vxt", bufs=3))
        vpool = ctx.enter_context(tc.tile_pool(name="vt", bufs=2))
        hpool = ctx.enter_context(tc.tile_pool(name="hpad", bufs=1))
        pspool = ctx.enter_context(tc.tile_pool(name="psum", bufs=4, space="PSUM"))
        tmppool = ctx.enter_context(tc.tile_pool(name="tmp", bufs=10))
        opool = ctx.enter_context(tc.tile_pool(name="out", bufs=3))

        w1_s = [
            wpool.tile([P, 3 * NPL * CB * P], F16, tag=f"w1_{cib}", name=f"w1{cib}")
            for cib in range(CB)
        ]
        w2_s = []
        # chunked per plane-group, q-major ACROSS cib so the first matmul's
        # weights (q=0, both cib) land first
        wchunk = 3 * CB * P
        for q in range(NPL):
            for cib in range(CB):
                sl = slice(q * wchunk, (q + 1) * wchunk)
                nc.scalar.dma_start(w1_s[cib][:, sl], w1_d[cib, :, sl])
        bn_s = wpool.tile([P, 4 * CB], F32, tag="bn", name="bn_s")
        nc.scalar.dma_start(bn_s[:], bn_d[:])
        for cib in range(CB):
            t2 = wpool.tile([P, 3 * NPL * CB * P], F16, tag=f"w2_{cib}", name=f"w2_{cib}")
            nc.scalar.dma_start(t2[:], w2_d[cib])
            w2_s.append(t2)

        def bnv(vec, cob):
            return bn_s[:, vec * CB + cob : vec * CB + cob + 1]

        # warmup matmuls (HAM) while DMAs land
        warm = wpool.tile([P, HALF], F16, tag="warm", name="warm")
        nc.vector.memset(warm[:], 0.0)
        warm_ps = pspool.tile([P, 1024], F32, tag="ps", name="warm_ps")
        n_warm = 22
        for i in range(n_warm):
            nc.tensor.matmul(
                warm_ps[:, 0:HALF], warm[:, 0:P], warm[:], start=(i == 0), stop=(i == n_warm - 1)
            )

        hslots = [
            hpool.tile([P, CB, PAD], F16, tag=f"hp{i}", name=f"hp{i}") for i in range(HR)
        ]
        for s in hslots:
            for cib in range(CB):
                h3 = s[:, cib].rearrange("p (r c) -> p r c", c=WP)
                nc.vector.memset(h3[:, 0 : HP : HP - 1, :], 0.0)
                nc.vector.memset(h3[:, 1 : HP - 1, 0 : WP : WP - 1], 0.0)

        xtiles, vxt, vht = {}, {}, {}

        def load_x(n):
            t = xpool.tile([P, CB, PAD], F16, tag="xp", name=f"xt_{n}")
            tv = vxpool.tile([P, CB, NPL * HP * TX], F16, tag="vx", name=f"vxt_{n}")
            vchunk = HP * TX
            # V(x) planes chunked in matmul consumption order (JORD), cib
            # interleaved so the first matmul pair's planes land first; the
            # residual x tiles (consumed late, by epi2) go last
            for j in JORD:
                for cib in range(CB):
                    sl = slice(j * vchunk, (j + 1) * vchunk)
                    nc.sync.dma_start(tv[:, cib, sl], vx_d[n, cib, :, sl])
            for cib in range(CB):
                nc.sync.dma_start(t[:, cib], x_d[n, cib])
            xtiles[n] = t
            vxt[n] = tv.rearrange("p b (j r c) -> p b j r c", j=NPL, c=TX)

        def in_tf(src, vdst, eng, eng2=None):
            """V planes from padded source [P, CB, PAD], emitted in MM
            consumption order (j=1,2,0,3) so conv can start after op 1."""
            e2 = eng2 or eng
            s4 = src.rearrange("p b (r c) -> p b r c", c=WP)
            xb = [s4[:, :, :, b : b + 2 * TX - 1 : 2] for b in range(4)]
            v = [vdst[:, :, j] for j in range(NPL)]
            eng.tensor_add(v[1], xb[1], xb[2])
            e2.tensor_sub(v[2], xb[2], xb[1])
            eng.tensor_sub(v[0], xb[0], xb[2])
            e2.tensor_sub(v[3], xb[1], xb[3])

        def make_v(n, store, src, eng, eng2=None):
            vt_ = vpool.tile([P, CB, NPL, HP, TX], F16, tag="v", name=f"v_{len(store)}_{n}")
            in_tf(src, vt_, eng, eng2)
            store[n] = vt_

        def conv_cob(ws, vt_, which, n, cob):
            """24 matmuls (N=512) for one cob; returns (tileA, tileB) two-bank
            tiles, each plane a contiguous [32y x 16tx] 512-f32 region that
            stays inside one PSUM bank."""
            pa = pspool.tile([P, 1024], F32, tag="ps", name=f"ps{which}A_{n}_{cob}")
            pb = pspool.tile([P, 1024], F32, tag="ps", name=f"ps{which}B_{n}_{cob}")
            tiles = {"A": pa, "B": pb}
            rA = None
            for j in JORD:
                key, off = PLANE_SLOT[j]
                q = JPOS[j]
                dst = tiles[key][:, 2 * off : 2 * off + 512]
                for cib in range(CB):
                    for dy in range(3):
                        w_ap = ws[cib][
                            :,
                            ((q * 3 + dy) * CB + cob) * P : ((q * 3 + dy) * CB + cob + 1) * P,
                        ]
                        rhs = vt_[:, cib, j, dy : dy + H, :]
                        nc.tensor.matmul(
                            dst,
                            w_ap,
                            rhs,
                            start=(cib == 0 and dy == 0),
                            stop=(cib == CB - 1 and dy == 2),
                        )
                if j == 2:
                    # tileA (M1, M2) complete: rA and dd (both read only
                    # tileA) hide under tileB's 12 matmuls.  u0 must NOT be
                    # hoisted: reading tileB mid-write serializes against the
                    # remaining matmuls at tile granularity.
                    rA = tmppool.tile(
                        [P, 2 * 16 * TX], F32, tag="rA", name=f"rA_{which}_{n}_{cob}"
                    )
                    nc.vector.reduce_sum(
                        rA[:],
                        pa[:].rearrange("p (j t) -> p t j", j=2),
                        axis=mybir.AxisListType.X,
                    )
                    dd = tmppool.tile(
                        [P, 2 * 16 * TX], F32, tag="dd", name=f"dd_{which}_{n}_{cob}"
                    )
                    nc.vector.scalar_tensor_tensor(
                        dd[:], pa[:, 512:1024], -2.0, rA[:],
                        op0=mybir.AluOpType.mult, op1=mybir.AluOpType.add,
                    )
            return pa, pb, rA, dd

        def combine(n, which, pb, rA, dd, cob):
            """u0=M0+rA and u1=dd-M3 after the matmuls (rA, dd were emitted
            inside conv_cob, hidden under tileB's matmuls)."""
            u0 = tmppool.tile([P, 2 * 16 * TX], F32, tag="u0", name=f"u0_{which}_{n}_{cob}")
            nc.vector.scalar_tensor_tensor(
                u0[:], pb[:, 0:512], 1.0, rA[:],
                op0=mybir.AluOpType.mult, op1=mybir.AluOpType.add,
            )
            u1 = tmppool.tile([P, 2 * 16 * TX], F32, tag="u1", name=f"u1_{which}_{n}_{cob}")
            nc.vector.scalar_tensor_tensor(
                u1[:], pb[:, 512:1024], -1.0, dd[:],
                op0=mybir.AluOpType.mult, op1=mybir.AluOpType.add,
            )
            return u0, u1

        def epi1_cob(n, cob, pb, rA, ddv):
            hdst = hslots[n % HR]
            h3 = hdst[:, cob].rearrange("p (r c) -> p r c", c=WP)
            u0, u1 = combine(n, 1, pb, rA, ddv, cob)
            for u, t in ((0, u0), (1, u1)):
                uv = t.rearrange("p (r q) -> p r q", q=TX)
                nc.scalar.activation(
                    h3[:, 1 : H + 1, 1 + u : 1 + u + 2 * TX - 1 : 2],
                    uv[:],
                    mybir.ActivationFunctionType.Relu,
                    bias=bnv(1, cob),
                    scale=bnv(0, cob),
                )

        def epi2_cob(n, cob, pb, rA, ddv):
            xsrc = xtiles[n]
            ot = opool.tile([P, H * W], F32, tag="ot", name=f"ot_{n}_{cob}")
            ov = ot.rearrange("p (r c) -> p r c", c=W)
            x3 = xsrc[:, cob].rearrange("p (r c) -> p r c", c=WP)
            u0, u1 = combine(n, 2, pb, rA, ddv, cob)
            for u, t in ((0, u0), (1, u1)):
                uv = t.rearrange("p (r q) -> p r q", q=TX)
                rr = tmppool.tile([P, 2 * 16 * TX], F32, tag="rr", name=f"rr_{n}_{cob}_{u}")
                rv = rr.rearrange("p (r q) -> p r q", q=TX)
                nc.vector.scalar_tensor_tensor(
                    rv[:],
                    uv[:],
                    bnv(2, cob),
                    x3[:, 1 : H + 1, 1 + u : 1 + u + 2 * TX - 1 : 2],
                    op0=mybir.AluOpType.mult,
                    op1=mybir.AluOpType.add,
                )
                nc.scalar.activation(
                    ov[:, :, u : u + 2 * TX - 1 : 2],
                    rv[:],
                    mybir.ActivationFunctionType.Relu,
                    bias=bnv(3, cob),
                    scale=1.0,
                )
            y3 = y_d[n, cob * P : (cob + 1) * P].rearrange("c h w -> c (h w)")
            for half in range(2):
                nc.sync.dma_start(
                    y3[:, half * HALF : (half + 1) * HALF],
                    ot[:, half * HALF : (half + 1) * HALF],
                )

        # ---- pipeline ----
        # engine program orders:
        #   PE:  conv1(0), conv1(1), conv2(0), conv1(2), conv2(1), ...
        #   DVE: epi1(0), epi1(1), epi2(0), epi1(2), epi2(1), ...
        # epi1(n+1) is emitted before conv2(n) so the PSUM slots conv2(n)
        # waits on are released by vector-engine work that is ahead of it.
        def conv1_and_epi1(n):
            for cob in range(CB):
                pa, pb, rA, ddv = conv_cob(w1_s, vxt[n], 1, n, cob)
                epi1_cob(n, cob, pb, rA, ddv)
            vxt.pop(n)

        def conv2_and_epi2(n):
            for cob in range(CB):
                pa, pb, rA, ddv = conv_cob(w2_s, vht[n], 2, n, cob)
                epi2_cob(n, cob, pb, rA, ddv)
            vht.pop(n)
            del xtiles[n]

        for n in range(min(2, nimg)):
            load_x(n)
        conv1_and_epi1(0)
        for n in range(nimg):
            make_v(n, vht, hslots[n % HR], nc.gpsimd)
            if n + 1 < nimg:
                conv1_and_epi1(n + 1)
            conv2_and_epi2(n)
            if n + 2 < nimg:
                load_x(n + 2)

    nc.compile()
    return nc


_NC_CACHE: dict = {}


def _get_nc(nimg: int = NIMG):
    if nimg not in _NC_CACHE:
        _NC_CACHE[nimg] = build(nimg)
    return _NC_CACHE[nimg]


_G = np.array(
    [[1, 0, 0], [0.5, 0.5, 0.5], [0.5, -0.5, 0.5], [0, 0, 1]], np.float32
)


def _prep_host(w1, g1, b1, rm1, rv1, w2, g2, b2, rm2, rv2):
    eps = 1e-5
    f = np.float32
    inv1 = (np.asarray(g1, f) / np.sqrt(np.asarray(rv1, f) + eps)).astype(f)
    b1p = (np.asarray(b1, f) - np.asarray(rm1, f) * inv1).astype(f)
    inv2 = (np.asarray(g2, f) / np.sqrt(np.asarray(rv2, f) + eps)).astype(f)
    b2p = (np.asarray(b2, f) - np.asarray(rm2, f) * inv2).astype(f)
    bnv = np.zeros((P, 4 * CB), f)
    for vi, v in enumerate([inv1, b1p, inv2, b2p]):
        for cob in range(CB):
            bnv[:, vi * CB + cob] = v[cob * P : (cob + 1) * P]

    def wt(w):
        w = np.asarray(w, f)
        wp = np.einsum("oidk,jk->oidj", w, _G)          # [o, i, dy, j]
        wp = wp.reshape(CB, P, CB, P, 3, NPL)            # [cob, co, cib, ci, dy, j]
        wp = wp[..., list(JORD)]                         # planes in consumption order
        wp = wp.transpose(2, 3, 5, 4, 0, 1)              # [cib, ci, q, dy, cob, co]
        return np.ascontiguousarray(
            wp.reshape(CB, P, 3 * NPL * CB * P).astype(np.float16)
        )

    return wt(w1), wt(w2), bnv


def _pad_x(x):
    n = x.shape[0]
    xp = np.zeros((n, C, HP, WP), np.float32)
    xp[:, :, 1 : H + 1, 1 : W + 1] = x
    return np.ascontiguousarray(xp.reshape(n, CB, P, PAD).astype(np.float16))


def _host_vx(x):
    """x-side F(2,3) input transform on host: [n,C,H,W] f32 ->
    [n, CB, P, NPL*HP*TX] fp16 in natural plane order (V0..V3)."""
    n = x.shape[0]
    xp = np.zeros((n, C, HP, WP), np.float16)
    xp[:, :, 1 : H + 1, 1 : W + 1] = x.astype(np.float16)
    tap = [xp[:, :, :, b : b + 2 * TX - 1 : 2].astype(np.float32) for b in range(4)]
    v = np.stack(
        [tap[0] - tap[2], tap[1] + tap[2], tap[2] - tap[1], tap[1] - tap[3]], axis=2
    ).astype(np.float16)  # [n, C, 4, HP, TX]
    return np.ascontiguousarray(v.reshape(n, CB, P, NPL * HP * TX))


def make_in_maps(x, w1, g1, b1, rm1, rv1, w2, g2, b2, rm2, rv2):
    x = np.asarray(x, np.float32)
    nimg = x.shape[0] // N_CORES
    w1t, w2t, bnv = _prep_host(w1, g1, b1, rm1, rv1, w2, g2, b2, rm2, rv2)
    return [
        {
            "xp": _pad_x(x[c * nimg : (c + 1) * nimg]),
            "vxp": _host_vx(x[c * nimg : (c + 1) * nimg]),
            "w1t": w1t,
            "w2t": w2t,
            "bnv": bnv,
        }
        for c in range(N_CORES)
    ]


def kernel(x, w1, g1, b1, rm1, rv1, w2, g2, b2, rm2, rv2):
    x = np.asarray(x, np.float32)
    assert x.shape[0] % N_CORES == 0
    nc = _get_nc(x.shape[0] // N_CORES)
    in_maps = make_in_maps(x, w1, g1, b1, rm1, rv1, w2, g2, b2, rm2, rv2)
    res = run_bass_kernel_spmd(nc, in_maps, list(range(N_CORES)))
    return np.ascontiguousarray(
        np.concatenate([res.results[c]["y"] for c in range(N_CORES)], axis=0)
    )

